# revision 1
# baseline (speedup 1.0000x reference)
"""MultiHeadedAttention block (B=4, S=2048, D=1024, H=16) on 8 TRN2 cores.

Sharding: core c handles batch b=c//2 and query-row half c%2 (1024 rows).
Each core computes full K/V projections for its batch (2x redundant within a
batch pair), attention for all 16 heads over its 1024 query rows, then
O-projection + residual + LayerNorm. No collectives.

Device layouts (per core):
  Q^T  [o=1024, r=1024]  feature-major (partitions = features), per-ot tiles
  K^T  [o, k] projected per head pair inside the attention loop (no spill)
  V    [k=2048, o=1024]  row-major per-rt tiles, with a ones column per head
  scores computed transposed: S_t[k, q] = K_h^T Q_h  (softmax along k =
  partitions; exp without max-subtraction is safe: |logits| < ~3).
  P@V with the ones-augmented V gives the softmax denominator as row DK;
  normalization multiplies by a DMA-broadcast reciprocal. The V bias is
  exact through the normalization (bv*denom/denom), so it is added
  per-partition after normalizing.
All matmuls run in float32r (full PE rate at moving dim >= 256).
"""

import sys

if "/opt/trn_rl_repo" not in sys.path:
    sys.path.insert(0, "/opt/trn_rl_repo")

import ml_dtypes
import numpy as np

import concourse.bass as bass
import concourse.mybir as mybir
import concourse.tile as tile
from concourse.bass_utils import run_bass_kernel_spmd

B, S, D, H, DK = 4, 2048, 1024, 16, 64
P = 128
M = S // 2          # query rows per core
NDT = D // P        # 8 contraction chunks
NOT = D // P        # 8 output-feature chunks (= head pairs)
NHP = H // 2        # 8 head pairs
NKT = S // P        # 16 key chunks
NQT = M // 512      # 2 query 512-chunks
NRT_K = S // 512    # 4 key-row 512-chunks
NRT_V = S // P      # 16 V row chunks
NRT_O = M // P      # 8 output row chunks
KG = 2              # k-chunks per exp group
F32 = mybir.dt.float32
MM_DT = mybir.dt.float32r
AF = mybir.ActivationFunctionType
ALU = mybir.AluOpType


def _split_sync_waits(nc, max_waits=1):
    """Split instructions carrying more than max_waits sem waits.

    The container's walrus rejects instructions with multiple sync wait
    commands, so excess waits move onto NoOp instructions inserted just
    before, on the same engine.
    """
    idx = 0
    for f in nc.m.functions:
        for blk in f.blocks:
            newl = []
            for inst in blk.instructions:
                si = inst.sync_info
                waits = list(si.on_wait) if si is not None and si.on_wait else []
                if len(waits) > max_waits:
                    extra = waits[max_waits:]
                    si.on_wait = waits[:max_waits]
                    for j in range(0, len(extra), max_waits):
                        nop = mybir.InstNoOp(name=f"I-wsplit-{idx}", ins=[], outs=[])
                        idx += 1
                        nop.engine = inst.engine
                        nop.sync_info = mybir.SyncInfo(
                            on_wait=extra[j : j + max_waits], on_update=[]
                        )
                        newl.append(nop)
                newl.append(inst)
            blk.instructions = newl


def build_nc(loops=0):
    nc = bass.Bass()
    xqT = nc.dram_tensor("xqT", [D, M], mybir.dt.bfloat16, kind="ExternalInput")
    xkT = nc.dram_tensor("xkT", [D, S], mybir.dt.bfloat16, kind="ExternalInput")
    xvT = nc.dram_tensor("xvT", [D, S], mybir.dt.bfloat16, kind="ExternalInput")
    qres = nc.dram_tensor("qres", [M, D], F32, kind="ExternalInput")
    WqT = nc.dram_tensor("WqT", [D, D], mybir.dt.bfloat16, kind="ExternalInput")
    WkT = nc.dram_tensor("WkT", [D, D], mybir.dt.bfloat16, kind="ExternalInput")
    WvT = nc.dram_tensor("WvT", [D, D], mybir.dt.bfloat16, kind="ExternalInput")
    WoT = nc.dram_tensor("WoT", [D, D], mybir.dt.bfloat16, kind="ExternalInput")
    bqv = nc.dram_tensor("bq", [D], F32, kind="ExternalInput")
    bkv = nc.dram_tensor("bk", [D], F32, kind="ExternalInput")
    bvv = nc.dram_tensor("bv", [D], F32, kind="ExternalInput")
    gv = nc.dram_tensor("ln_g", [D], F32, kind="ExternalInput")
    bv2 = nc.dram_tensor("ln_b", [D], F32, kind="ExternalInput")
    onesv = nc.dram_tensor("onesv", [P, NRT_V * H], mybir.dt.bfloat16, kind="ExternalInput")
    onesf = nc.dram_tensor("onesf", [DK], F32, kind="ExternalInput")
    out = nc.dram_tensor("out", [M, D], F32, kind="ExternalOutput")

    WqT_r = WqT[:, :].rearrange("(a p) o -> p a o", p=P)
    WkT_r = WkT[:, :].rearrange("(a p) o -> p a o", p=P)
    WvT_r = WvT[:, :].rearrange("(a p) o -> p a o", p=P)
    WoT_r = WoT[:, :].rearrange("(a p) o -> p a o", p=P)
    xqT_r = xqT[:, :].rearrange("(a p) r -> p a r", p=P)
    xkT_r = xkT[:, :].rearrange("(a p) r -> p a r", p=P)
    xvT_r = xvT[:, :].rearrange("(a p) r -> p a r", p=P)

    import contextlib

    with tile.TileContext(nc) as tc:
        loop_cm = tc.For_i(0, loops, 1) if loops else contextlib.nullcontext()
        loop_cm.__enter__()
        pxo_cm = tc.tile_pool(name="pxo", bufs=1)
        pxo = pxo_cm.__enter__()
        with (
            tc.tile_pool(name="pqv", bufs=1) as pqv,
        ):
            XO = [
                pxo.tile([P, M], mybir.dt.bfloat16, tag=f"XO{i}", name=f"XO{i}")
                for i in range(NHP)
            ]

            QT = []
            for ot in range(NOT):
                t = pqv.tile([P, M], mybir.dt.bfloat16, tag=f"QT{ot}", name=f"QT{ot}")
                QT.append(t)
            bq_p = pqv.tile([P, NOT], F32)
            bk_p = pqv.tile([P, NOT], F32)
            bv_p = pqv.tile([P, NOT], F32)
            nc.gpsimd.dma_start(bq_p, bqv[:].rearrange("(a p) -> p a", p=P))
            nc.gpsimd.dma_start(bk_p, bkv[:].rearrange("(a p) -> p a", p=P))
            nc.gpsimd.dma_start(bv_p, bvv[:].rearrange("(a p) -> p a", p=P))
            Vt = []
            for rt in range(NRT_V):
                t = pqv.tile([P, H, DK + 1], mybir.dt.bfloat16, tag=f"Vt{rt}", name=f"Vt{rt}")
                nc.gpsimd.dma_start(
                    t[:, :, DK : DK + 1],
                    onesv[:, rt * H : (rt + 1) * H],
                )
                Vt.append(t)
            ones_t = pqv.tile([1, DK], MM_DT)
            nc.gpsimd.dma_start(
                ones_t, onesf[:].partition_broadcast(1).bitcast(MM_DT)
            )

            # wv loads early so phase B starts without a DMA stall
            pwv_cm = tc.tile_pool(name="pwv", bufs=NDT, side="right")
            pwv = pwv_cm.__enter__()
            wv = []
            for dt in range(NDT):
                w_t = pwv.tile([P, D], mybir.dt.bfloat16, tag="wv", name=f"wv{dt}")
                nc.gpsimd.dma_start(w_t, WvT_r[:, dt, :])
                wv.append(w_t)

            pbx_cm = tc.tile_pool(name="pbx", bufs=3, side="right")
            pbx = pbx_cm.__enter__()
            psAB_cm = tc.tile_pool(name="psAB", bufs=8, space="PSUM")
            psAB = psAB_cm.__enter__()

            # ---- Phase A: Q^T = (Wq/8) @ x_q^T + bq/8, layout [o, r]
            with (
                tc.tile_pool(name="pa", bufs=NDT) as pa,
            ):
                wq = []
                xq = []
                xv_pre = {}
                for dt in range(NDT):
                    w_t = pa.tile([P, D], mybir.dt.bfloat16, tag="wq", name=f"wq{dt}")
                    nc.sync.dma_start(w_t, WqT_r[:, dt, :])
                    wq.append(w_t)
                    x_t = pa.tile([P, M], mybir.dt.bfloat16, tag="xq", name=f"xq{dt}")
                    nc.sync.dma_start(x_t, xqT_r[:, dt, :])
                    xq.append(x_t)
                    if dt in (2, 4, 6):
                        rt = dt // 2 - 1
                        xv_t = pbx.tile(
                            [P, NDT, P], mybir.dt.bfloat16, tag="xv", name="xv"
                        )
                        nc.sync.dma_start(
                            xv_t, xvT_r[:, :, rt * P : (rt + 1) * P]
                        )
                        xv_pre[rt] = xv_t
                for ot in range(NOT):
                    for qt in range(NQT):
                        ps = psAB.tile([P, 512], F32, tag='ps', name='ps')
                        for dt in range(NDT):
                            nc.tensor.matmul(
                                ps,
                                wq[dt][:, ot * P : (ot + 1) * P],
                                xq[dt][:, qt * 512 : (qt + 1) * 512],
                                start=(dt == 0),
                                stop=(dt == NDT - 1),
                            )
                        nc.vector.tensor_scalar_add(
                            QT[ot][:, qt * 512 : (qt + 1) * 512],
                            ps,
                            bq_p[:, ot : ot + 1],
                        )

            # xk loads during phase B so phase D starts without a DMA stall
            pdx_cm = tc.tile_pool(name="pdx", bufs=NDT)
            pdx = pdx_cm.__enter__()
            xk = []
            for dt in range(NDT):
                x_t = pdx.tile([P, S], mybir.dt.bfloat16, tag="xk", name=f"xk{dt}")
                nc.gpsimd.dma_start(x_t, xkT_r[:, dt, :])
                xk.append(x_t)

            # ---- Phase B: V = x_v @ Wv^T (bias folded in later), [r, o]
            if True:
                for rt in range(NRT_V):
                    if rt in xv_pre:
                        xv = xv_pre.pop(rt)
                    else:
                        xv = pbx.tile(
                            [P, NDT, P], mybir.dt.bfloat16, tag="xv", name="xv"
                        )
                        veng = nc.sync if rt < 6 else nc.gpsimd
                        veng.dma_start(xv, xvT_r[:, :, rt * P : (rt + 1) * P])
                    for o2 in range(2):
                        ps = psAB.tile([P, 512], F32, tag='ps', name='ps')
                        for dt in range(NDT):
                            nc.tensor.matmul(
                                ps,
                                xv[:, dt, :],
                                wv[dt][:, o2 * 512 : (o2 + 1) * 512],
                                start=(dt == 0),
                                stop=(dt == NDT - 1),
                            )
                        nc.vector.tensor_copy(
                            Vt[rt][:, o2 * 8 : (o2 + 1) * 8, 0:DK],
                            ps[:, :].rearrange("p (h e) -> p h e", e=DK),
                        )

            pbx_cm.__exit__(None, None, None)
            pwv_cm.__exit__(None, None, None)
            psAB_cm.__exit__(None, None, None)

            # wo prefetch during D so phase E starts without a DMA stall
            pwo_cm = tc.tile_pool(name="pwo", bufs=NDT, side="right")
            pwo = pwo_cm.__enter__()
            wo = []
            for dt in range(NDT):
                w_t = pwo.tile([P, D], mybir.dt.bfloat16, tag="wo", name=f"wo{dt}")
                nc.gpsimd.dma_start(w_t, WoT_r[:, dt, :])
                wo.append(w_t)

            # ---- Phase D: K^T projection fused with attention, per head pair
            with (
                tc.tile_pool(name="pdw", bufs=2) as pdw,
                tc.tile_pool(name="pdkt", bufs=2) as pdkt,
                tc.tile_pool(name="pde", bufs=2) as pde,
                tc.tile_pool(name="pdr", bufs=1) as pdr,
                tc.tile_pool(name="psS", bufs=1, space="PSUM") as psS,
                tc.tile_pool(name="psK", bufs=1, space="PSUM") as psK,
                tc.tile_pool(name="psR", bufs=1, space="PSUM") as psR,
                tc.tile_pool(name="psPV", bufs=1, space="PSUM") as psPV,
            ):
                kts = {}

                def kproj(hp):
                    wk = pdw.tile([P, NDT, P], mybir.dt.bfloat16, tag="wk", name="wk")
                    nc.sync.dma_start(wk, WkT_r[:, :, hp * P : (hp + 1) * P])
                    kt_t = pdkt.tile([P, S], mybir.dt.bfloat16, tag="kt", name="kt")
                    for rt in range(NRT_K):
                        ps = psK.tile([P, 512], F32, tag="kps", name="kps")
                        for dt in range(NDT):
                            nc.tensor.matmul(
                                ps,
                                wk[:, dt, :],
                                xk[dt][:, rt * 512 : (rt + 1) * 512],
                                start=(dt == 0),
                                stop=(dt == NDT - 1),
                            )
                        nc.vector.tensor_scalar_add(
                            kt_t[:, rt * 512 : (rt + 1) * 512],
                            ps,
                            bk_p[:, hp : hp + 1],
                        )
                    kts[hp] = kt_t

                def attn(hp):
                    kt_t = kts.pop(hp)
                    xo_t = XO[hp]
                    for qt in range(NQT):
                        pv = [
                            psPV.tile(
                                [DK + 1, 512], F32, tag=f"pv{h01}", name=f"pv{h01}"
                            )
                            for h01 in range(2)
                        ]
                        for ktg in range(NKT // KG):
                            sss = [
                                psS.tile(
                                    [P, KG, 512], F32, tag=f"ss{h01}", name=f"ss{h01}"
                                )
                                for h01 in range(2)
                            ]
                            for j in range(KG):
                                kt = ktg * KG + j
                                for h01 in range(2):
                                    pb_ = h01 * DK
                                    nc.tensor.matmul(
                                        sss[h01][:, j, :],
                                        kt_t[pb_ : pb_ + DK, kt * P : (kt + 1) * P],
                                        QT[hp][
                                            pb_ : pb_ + DK,
                                            qt * 512 : (qt + 1) * 512,
                                        ],
                                        start=True,
                                        stop=True,
                                        tile_position=(pb_, 0),
                                    )
                            exs = []
                            for h01 in range(2):
                                ex = pde.tile(
                                    [P, KG, 512],
                                    mybir.dt.bfloat16,
                                    tag=f"ex{h01}",
                                    name=f"ex{h01}",
                                )
                                nc.scalar.activation(ex, sss[h01], AF.Exp)
                                exs.append(ex)
                            for h01 in range(2):
                                for j in range(KG):
                                    kt = ktg * KG + j
                                    nc.tensor.matmul(
                                        pv[h01],
                                        Vt[kt][:, 2 * hp + h01, :],
                                        exs[h01][:, j, :],
                                        start=(kt == 0),
                                        stop=(kt == NKT - 1),
                                    )
                        for h01 in range(2):
                            pb_ = h01 * DK
                            rc = pdr.tile([1, 512], MM_DT, tag="rc", name="rc")
                            with nc.allow_low_precision(
                                reason="1/denom feeds f32r broadcast matmul"
                            ):
                                nc.vector.reciprocal(rc, pv[h01][DK : DK + 1, :])
                            rbp = psR.tile([DK, 512], F32, tag="rbp", name="rbp")
                            nc.tensor.matmul(rbp, ones_t, rc, start=True, stop=True)
                            dst = xo_t[pb_ : pb_ + DK, qt * 512 : (qt + 1) * 512]
                            nc.vector.tensor_copy(dst, pv[h01][0:DK, :])
                            nc.vector.tensor_mul(dst, dst, rbp)
                            nc.vector.tensor_scalar_add(
                                dst, dst, bv_p[pb_ : pb_ + DK, hp : hp + 1]
                            )

                kproj(0)
                for hp in range(NHP):
                    if hp + 1 < NHP:
                        kproj(hp + 1)
                    attn(hp)

            pdx_cm.__exit__(None, None, None)

        # ---- Phase E: out = LN(x_o @ Wo^T + bo + q)  (bo pre-added to qres)
        with (
            tc.tile_pool(name="pe1", bufs=NDT) as pe1,
            tc.tile_pool(name="pec", bufs=1) as pec,
            tc.tile_pool(name="peq", bufs=8) as peq,
            tc.tile_pool(name="pey", bufs=6) as pey,
            tc.tile_pool(name="pst", bufs=8) as pst,
            tc.tile_pool(name="psE", bufs=6, space="PSUM") as psE,
        ):
            g_b = pec.tile([P, D], F32)
            b_b = pec.tile([P, D], F32)
            eps_t = pec.tile([P, 1], F32)
            nc.sync.dma_start(g_b, gv[:].partition_broadcast(P))
            nc.sync.dma_start(b_b, bv2[:].partition_broadcast(P))
            nc.vector.memset(eps_t, 1e-5)
            xo = XO
            for rt in range(NRT_O):
                qr = peq.tile([P, D], F32)
                nc.gpsimd.dma_start(qr, qres[rt * P : (rt + 1) * P, :])
                y = pey.tile([P, D], F32)
                for o2 in range(2):
                    ps = psE.tile([P, 512], F32)
                    for hp in range(NOT):
                        nc.tensor.matmul(
                            ps,
                            xo[hp][:, rt * P : (rt + 1) * P],
                            wo[hp][:, o2 * 512 : (o2 + 1) * 512],
                            start=(hp == 0),
                            stop=(hp == NOT - 1),
                        )
                    nc.vector.tensor_add(
                        y[:, o2 * 512 : (o2 + 1) * 512],
                        ps,
                        qr[:, o2 * 512 : (o2 + 1) * 512],
                    )
                stats = pst.tile([P, 2, 6], F32)
                for sg in range(2):
                    nc.vector.bn_stats(
                        stats[:, sg, :], y[:, sg * 512 : (sg + 1) * 512]
                    )
                mv = pst.tile([P, 2], F32)
                nc.vector.bn_aggr(mv, stats)
                std = pst.tile([P, 1], F32)
                nc.scalar.activation(std, mv[:, 1:2], AF.Sqrt, bias=eps_t)
                rstd = pst.tile([P, 1], F32)
                nc.vector.reciprocal(rstd, std)
                nc.vector.tensor_scalar(
                    y,
                    y,
                    mv[:, 0:1],
                    rstd,
                    op0=ALU.subtract,
                    op1=ALU.mult,
                )
                eng = nc.vector if rt % 2 == 0 else nc.gpsimd
                eng.tensor_mul(y, y, g_b)
                eng.tensor_add(y, y, b_b)
                nc.sync.dma_start(out[rt * P : (rt + 1) * P, :], y)
        pwo_cm.__exit__(None, None, None)
        pxo_cm.__exit__(None, None, None)
        loop_cm.__exit__(None, None, None)
    _split_sync_waits(nc)
    return nc


_NC = None


def _get_nc():
    global _NC
    if _NC is None:
        _NC = build_nc()
    return _NC


def prepare_in_maps(q, k, v, Wq, bq, Wk, bk, Wv, bv, Wo, bo, ln_g, ln_b):
    f = np.float32
    q = np.asarray(q, f)
    k = np.asarray(k, f)
    v = np.asarray(v, f)
    scale = 1.0 / np.sqrt(np.float32(DK))
    WqT = np.ascontiguousarray((np.asarray(Wq, f).T * scale).astype(ml_dtypes.bfloat16))
    WkT = np.ascontiguousarray(np.asarray(Wk, f).T.astype(ml_dtypes.bfloat16))
    WvT = np.ascontiguousarray(np.asarray(Wv, f).T.astype(ml_dtypes.bfloat16))
    WoT = np.ascontiguousarray(np.asarray(Wo, f).T.astype(ml_dtypes.bfloat16))
    bq_s = np.asarray(bq, f) * scale
    common = {
        "WqT": WqT,
        "WkT": WkT,
        "WvT": WvT,
        "WoT": WoT,
        "bq": bq_s,
        "bk": np.asarray(bk, f),
        "bv": np.asarray(bv, f),
        "ln_g": np.asarray(ln_g, f),
        "ln_b": np.asarray(ln_b, f),
        "onesv": np.ones((P, NRT_V * H), ml_dtypes.bfloat16),
        "onesf": np.ones(DK, np.float32),
    }
    in_maps = []
    for c in range(8):
        b_, half = divmod(c, 2)
        qs = q[b_, half * M : (half + 1) * M, :]
        qres_c = qs + np.asarray(bo, f)[None, :]
        in_maps.append(
            dict(
                common,
                xqT=np.ascontiguousarray(qs.T.astype(ml_dtypes.bfloat16)),
                xkT=np.ascontiguousarray(k[b_].T.astype(ml_dtypes.bfloat16)),
                xvT=np.ascontiguousarray(v[b_].T.astype(ml_dtypes.bfloat16)),
                qres=np.ascontiguousarray(qres_c),
            )
        )
    return in_maps


def kernel(q, k, v, Wq, bq, Wk, bk, Wv, bv, Wo, bo, ln_g, ln_b):
    nc = _get_nc()
    in_maps = prepare_in_maps(q, k, v, Wq, bq, Wk, bk, Wv, bv, Wo, bo, ln_g, ln_b)
    res = run_bass_kernel_spmd(nc, in_maps, core_ids=list(range(8)))
    out = np.empty((B, S, D), np.float32)
    for c in range(8):
        b_, half = divmod(c, 2)
        out[b_, half * M : (half + 1) * M, :] = res.results[c]["out"]
    return out



# revision 5
# speedup vs baseline: 2.1546x; 2.1546x over previous
"""MultiHeadedAttention block (B=4, S=2048, D=1024, H=16) on 8 TRN2 cores.

Sharding: core c handles batch b=c//2 and query-row half c%2 (1024 rows).
Each core computes full K/V projections for its batch (2x redundant within a
batch pair), attention for all 16 heads over its 1024 query rows, then
O-projection + residual + LayerNorm. No collectives.

All four projections and the QK^T scores run in fp8e4m3 with the DoubleRow
perf mode (2x PE rate). Scale management: weights are stored as W.T*32 in
fp8, activations x in fp8, so Q'=K'=32(xW+b) (stored fp8 in a split-dk
[32p, 2, .] layout for DoubleRow scores), V'=32(xWv+bv) (bf16). Raw scores
are 1024*(QK^T); the softmax exp applies scale 1/8192 = 1/(1024*sqrt(64)).
The ones-column of V gives the softmax denominator D via the PV matmul; the
reciprocal is broadcast with a ones(=2.0) stationary vector so
xo = pv * (2/D) = 64*(attn_out + bv) in fp8, and phase E folds the
1/(64*32) into a 1/2048 multiply before the residual add + LayerNorm.

The softmax exp is split across three engines: exact Exp on the Activation
engine plus the int16-bitcast approximation exp(x) ~= bitcast_bf16(
int16(x*128*log2e + 127*128 - 5.5)) on Pool and DVE (max ~3% weight error,
negligible through the diffuse softmax at this tolerance).
"""

import sys

if "/opt/trn_rl_repo" not in sys.path:
    sys.path.insert(0, "/opt/trn_rl_repo")

import ml_dtypes
import numpy as np

import concourse.bass as bass
import concourse.mybir as mybir
import concourse.tile as tile
from concourse.bass_utils import run_bass_kernel_spmd

B, S, D, H, DK = 4, 2048, 1024, 16, 64
P = 128
M = S // 2          # query rows per core
NDT = D // P        # 8 contraction chunks of 128
NOT = D // P        # 8 output-feature chunks (= head pairs)
NHP = H // 2        # 8 head pairs
NKT = S // P        # 16 key chunks of 128
NQT = M // 512      # 2 query 512-chunks
NRT_K = S // 512    # 4 key-row 512-chunks
NRT_V = S // P      # 16 V row chunks
NRT_O = M // P      # 8 output row chunks
KG = 2              # k-chunks per exp group
NKG = NKT // KG     # 8 exp groups per (head, qt)
F32 = mybir.dt.float32
F8 = mybir.dt.float8e4
BF16 = mybir.dt.bfloat16
I16 = mybir.dt.int16
MM_DT = mybir.dt.float32r
AF = mybir.ActivationFunctionType
ALU = mybir.AluOpType
DR = mybir.MatmulPerfMode.DoubleRow

LOG2E = 1.4426950408889634
EXP_SCALE = 1.0 / 8192.0                 # 1/(32*32*sqrt(DK))
EXP_A = 128.0 * LOG2E * EXP_SCALE        # int16-bitcast exp multiplier
EXP_B = 127.0 * 128.0 - 5.5              # exponent bias - mean sawtooth corr
# exp engine rotation per (head, qt): Pool cannot read PSUM, so split
# between Activation (exact) and DVE (bitcast approx) only
EXP_ENGINES = "AADAAADA"


def _split_sync_waits(nc, max_waits=1):
    """Split instructions carrying more than max_waits sem waits.

    The container's walrus rejects instructions with multiple sync wait
    commands, so excess waits move onto NoOp instructions inserted just
    before, on the same engine.
    """
    idx = 0
    for f in nc.m.functions:
        for blk in f.blocks:
            newl = []
            for inst in blk.instructions:
                si = inst.sync_info
                waits = list(si.on_wait) if si is not None and si.on_wait else []
                if len(waits) > max_waits:
                    extra = waits[max_waits:]
                    si.on_wait = waits[:max_waits]
                    for j in range(0, len(extra), max_waits):
                        nop = mybir.InstNoOp(name=f"I-wsplit-{idx}", ins=[], outs=[])
                        idx += 1
                        nop.engine = inst.engine
                        nop.sync_info = mybir.SyncInfo(
                            on_wait=extra[j : j + max_waits], on_update=[]
                        )
                        newl.append(nop)
                newl.append(inst)
            blk.instructions = newl


def build_nc(loops=0):
    nc = bass.Bass()
    xq8 = nc.dram_tensor("xq8", [DK, NDT, 2, M], F8, kind="ExternalInput")
    xk8 = nc.dram_tensor("xk8", [DK, NDT, 2, S], F8, kind="ExternalInput")
    xv8 = nc.dram_tensor("xv8", [DK, NDT, 2, S], F8, kind="ExternalInput")
    wq8 = nc.dram_tensor("wq8", [DK, NDT, 2, D], F8, kind="ExternalInput")
    wk8 = nc.dram_tensor("wk8", [DK, NDT, 2, D], F8, kind="ExternalInput")
    wv8 = nc.dram_tensor("wv8", [DK, NDT, 2, D], F8, kind="ExternalInput")
    wo8 = nc.dram_tensor("wo8", [DK, NDT, 2, D], F8, kind="ExternalInput")
    bq32 = nc.dram_tensor("bq32", [P, NOT], F32, kind="ExternalInput")
    bk32 = nc.dram_tensor("bk32", [P, NOT], F32, kind="ExternalInput")
    bvb = nc.dram_tensor("bvb", [D], BF16, kind="ExternalInput")
    qres = nc.dram_tensor("qres", [M, D], BF16, kind="ExternalInput")
    gv = nc.dram_tensor("ln_g", [D], F32, kind="ExternalInput")
    bv2 = nc.dram_tensor("ln_b", [D], F32, kind="ExternalInput")
    onesv = nc.dram_tensor("onesv", [P, NRT_V * H], BF16, kind="ExternalInput")
    ones2 = nc.dram_tensor("ones2", [DK], F32, kind="ExternalInput")
    out = nc.dram_tensor("out", [M, D], F32, kind="ExternalOutput")

    import contextlib

    with tile.TileContext(nc) as tc:
        loop_cm = tc.For_i(0, loops, 1) if loops else contextlib.nullcontext()
        loop_cm.__enter__()
        pxo_cm = tc.tile_pool(name="pxo", bufs=1)
        pxo = pxo_cm.__enter__()
        with (
            tc.tile_pool(name="pqv", bufs=1) as pqv,
        ):
            # attention outputs, fp8 split-head-pair layout for O-proj DR
            XO = [
                pxo.tile([DK, 2, M], F8, tag=f"XO{i}", name=f"XO{i}")
                for i in range(NHP)
            ]

            # Q' fp8, 2 heads across partitions: head h at partition
            # (h%2)*64, slot h//2, sub i = dk//32
            QT = pqv.tile([P, 8, 2, M], F8, tag="QT", name="QT")
            bq_p = pqv.tile([P, NOT], F32)
            bk_p = pqv.tile([P, NOT], F32)
            bv_b = pqv.tile([P, D], BF16)
            nc.gpsimd.dma_start(bq_p, bq32[:, :])
            nc.gpsimd.dma_start(bk_p, bk32[:, :])
            nc.gpsimd.dma_start(bv_b, bvb[:].partition_broadcast(P))
            Vt = []
            for rt in range(NRT_V):
                t = pqv.tile([P, H, DK + 1], BF16, tag=f"Vt{rt}", name=f"Vt{rt}")
                nc.gpsimd.dma_start(
                    t[:, :, DK : DK + 1],
                    onesv[:, rt * H : (rt + 1) * H],
                )
                Vt.append(t)
            ones_t = pqv.tile([1, DK], MM_DT)
            nc.gpsimd.dma_start(
                ones_t, ones2[:].partition_broadcast(1).bitcast(MM_DT)
            )

            # wv/xv load early so phase B starts without a DMA stall
            pwv_cm = tc.tile_pool(name="pwv", bufs=1, side="right")
            pwv = pwv_cm.__enter__()
            wv = pwv.tile([DK, NDT, 2, D], F8, tag="wv", name="wv")
            for dt in range(NDT):
                nc.gpsimd.dma_start(wv[:, dt, :, :], wv8[:, dt, :, :])

            pbx_cm = tc.tile_pool(name="pbx", bufs=1, side="right")
            pbx = pbx_cm.__enter__()
            xv = pbx.tile([DK, NDT, 2, S], F8, tag="xv", name="xv")

            psAB_cm = tc.tile_pool(name="psAB", bufs=4, space="PSUM")
            psAB = psAB_cm.__enter__()
            psg_cm = tc.tile_pool(name="psg", bufs=4)
            psg = psg_cm.__enter__()

            # ---- Phase A: Q' = 32*(Wq @ x_q^T) + 32 bq, fp8 DR
            with (
                tc.tile_pool(name="pa", bufs=1) as pa,
            ):
                wq = pa.tile([DK, NDT, 2, D], F8, tag="wq", name="wq")
                xq = pa.tile([DK, NDT, 2, M], F8, tag="xq", name="xq")
                for dt in range(NDT):
                    nc.sync.dma_start(wq[:, dt, :, :], wq8[:, dt, :, :])
                    nc.sync.dma_start(xq[:, dt, :, :], xq8[:, dt, :, :])
                    if dt in (2, 5):
                        # prefetch xv halves during A
                        h = 0 if dt == 2 else 1
                        for d2 in range(h * 4, h * 4 + 4):
                            nc.sync.dma_start(xv[:, d2, :, :], xv8[:, d2, :, :])
                for ot in range(NOT):
                    for qt in range(NQT):
                        ps = psAB.tile([P, 512], F32, tag="ps", name="ps")
                        for dt in range(NDT):
                            nc.tensor.matmul(
                                ps,
                                wq[:, dt, :, ot * P : (ot + 1) * P],
                                xq[:, dt, :, qt * 512 : (qt + 1) * 512],
                                start=(dt == 0),
                                stop=(dt == NDT - 1),
                                perf_mode=DR,
                            )
                        # bias-add + fp8 convert, then repack into the
                        # split-dk 2-heads-across-partitions layout via DMA
                        stage = psg.tile([P, 512], F8, tag="stg", name="stg")
                        nc.vector.tensor_scalar_add(
                            stage, ps, bq_p[:, ot : ot + 1]
                        )
                        for h01 in range(2):
                            h = 2 * ot + h01
                            for i in range(2):
                                pb = h01 * DK + i * 32
                                nc.sync.dma_start(
                                    QT[
                                        (h % 2) * 64 : (h % 2) * 64 + 32,
                                        h // 2,
                                        i,
                                        qt * 512 : (qt + 1) * 512,
                                    ],
                                    stage[pb : pb + 32, :],
                                )

            # xk/wk load during phase B so phase D starts without a DMA stall
            pdx_cm = tc.tile_pool(name="pdx", bufs=1)
            pdx = pdx_cm.__enter__()
            xk = pdx.tile([DK, NDT, 2, S], F8, tag="xk", name="xk")
            wk = pdx.tile([DK, NDT, 2, D], F8, tag="wk", name="wk")
            for dt in range(NDT):
                nc.gpsimd.dma_start(xk[:, dt, :, :], xk8[:, dt, :, :])
                nc.gpsimd.dma_start(wk[:, dt, :, :], wk8[:, dt, :, :])

            # ---- Phase B: V' = 32*(x_v @ Wv^T + bv), fp8 DR, bf16 out
            for rt in range(NRT_V):
                for o2 in range(2):
                    ps = psAB.tile([P, 512], F32, tag="ps", name="ps")
                    for dt in range(NDT):
                        nc.tensor.matmul(
                            ps,
                            xv[:, dt, :, rt * P : (rt + 1) * P],
                            wv[:, dt, :, o2 * 512 : (o2 + 1) * 512],
                            start=(dt == 0),
                            stop=(dt == NDT - 1),
                            perf_mode=DR,
                        )
                    nc.vector.tensor_tensor(
                        Vt[rt][:, o2 * 8 : (o2 + 1) * 8, 0:DK],
                        ps[:, :].rearrange("p (h e) -> p h e", e=DK),
                        bv_b[:, o2 * 512 : (o2 + 1) * 512].rearrange(
                            "p (h e) -> p h e", e=DK
                        ),
                        op=ALU.add,
                    )

            pbx_cm.__exit__(None, None, None)
            pwv_cm.__exit__(None, None, None)
            psAB_cm.__exit__(None, None, None)

            # wo prefetch during D so phase E starts without a DMA stall
            pwo_cm = tc.tile_pool(name="pwo", bufs=1, side="right")
            pwo = pwo_cm.__enter__()
            wo = pwo.tile([DK, NDT, 2, D], F8, tag="wo", name="wo")
            for dt in range(NDT):
                nc.gpsimd.dma_start(wo[:, dt, :, :], wo8[:, dt, :, :])

            # ---- Phase D: K' projection fused with attention
            with (
                tc.tile_pool(name="pdkt", bufs=1) as pdkt,
                tc.tile_pool(name="pde", bufs=3) as pde,
                tc.tile_pool(name="pdr", bufs=2) as pdr,
                tc.tile_pool(name="psS", bufs=2, space="PSUM") as psS,
                tc.tile_pool(name="psK", bufs=1, space="PSUM") as psK,
                tc.tile_pool(name="psPV", bufs=2, space="PSUM") as psPV,
                tc.tile_pool(name="psR", bufs=1, space="PSUM") as psR,
            ):
                # K' fp8, same 2-heads-across-partitions layout as QT
                KT = pdkt.tile([P, 8, 2, S], F8, tag="KT", name="KT")

                def kproj(hp):
                    for rt in range(NRT_K):
                        ps = psK.tile([P, 512], F32, tag="kps", name="kps")
                        for dt in range(NDT):
                            nc.tensor.matmul(
                                ps,
                                wk[:, dt, :, hp * P : (hp + 1) * P],
                                xk[:, dt, :, rt * 512 : (rt + 1) * 512],
                                start=(dt == 0),
                                stop=(dt == NDT - 1),
                                perf_mode=DR,
                            )
                        stage = psg.tile([P, 512], F8, tag="stg", name="stg")
                        nc.vector.tensor_scalar_add(
                            stage, ps, bk_p[:, hp : hp + 1]
                        )
                        for h01 in range(2):
                            h = 2 * hp + h01
                            for i in range(2):
                                pb = h01 * DK + i * 32
                                nc.sync.dma_start(
                                    KT[
                                        (h % 2) * 64 : (h % 2) * 64 + 32,
                                        h // 2,
                                        i,
                                        rt * 512 : (rt + 1) * 512,
                                    ],
                                    stage[pb : pb + 32, :],
                                )

                def attn(h):
                    hp, h01 = divmod(h, 2)
                    xo_t = XO[hp]
                    kb = (h % 2) * 64
                    hs = h // 2
                    for qt in range(NQT):
                        pv = psPV.tile([DK + 1, 512], F32, tag="pv", name="pv")
                        for ktg in range(NKG):
                            ss = psS.tile([P, KG, 512], F32, tag="ss", name="ss")
                            for j in range(KG):
                                kt = ktg * KG + j
                                nc.tensor.matmul(
                                    ss[:, j, :],
                                    KT[kb : kb + 32, hs, :, kt * P : (kt + 1) * P],
                                    QT[kb : kb + 32, hs, :, qt * 512 : (qt + 1) * 512],
                                    start=True,
                                    stop=True,
                                    perf_mode=DR,
                                )
                            ex = pde.tile([P, KG, 512], I16, tag="ex", name="ex")
                            ecode = EXP_ENGINES[ktg]
                            if ecode == "A":
                                nc.scalar.activation(
                                    ex[:, :, :].bitcast(BF16),
                                    ss,
                                    AF.Exp,
                                    scale=EXP_SCALE,
                                )
                            else:
                                nc.vector.tensor_scalar(
                                    ex,
                                    ss,
                                    EXP_A,
                                    EXP_B,
                                    op0=ALU.mult,
                                    op1=ALU.add,
                                )
                            for j in range(KG):
                                kt = ktg * KG + j
                                nc.tensor.matmul(
                                    pv,
                                    Vt[kt][:, h, :],
                                    ex[:, j, :].bitcast(BF16),
                                    start=(kt == 0),
                                    stop=(kt == NKT - 1),
                                )
                        rc = pdr.tile([1, 512], MM_DT, tag="rc", name="rc")
                        with nc.allow_low_precision(
                            reason="2/denom feeds f32r broadcast matmul"
                        ):
                            nc.vector.reciprocal(rc, pv[DK : DK + 1, :])
                        rbp = psR.tile([DK, 512], F32, tag="rbp", name="rbp")
                        nc.tensor.matmul(rbp, ones_t, rc, start=True, stop=True)
                        pvs = pdr.tile([DK, 512], BF16, tag="pvs", name="pvs")
                        nc.vector.tensor_copy(pvs, pv[0:DK, :])
                        nc.vector.tensor_tensor(
                            xo_t[:, h01, qt * 512 : (qt + 1) * 512],
                            pvs,
                            rbp,
                            op=ALU.mult,
                        )

                kproj(0)
                for hp in range(NHP):
                    if hp + 1 < NHP:
                        kproj(hp + 1)
                    attn(2 * hp)
                    attn(2 * hp + 1)

            pdx_cm.__exit__(None, None, None)
            psg_cm.__exit__(None, None, None)

        # ---- Phase E: out = LN(x_o @ Wo^T + bo + q)  (bo pre-added to qres)
        with (
            tc.tile_pool(name="pec", bufs=1) as pec,
            tc.tile_pool(name="peq", bufs=4) as peq,
            tc.tile_pool(name="pey", bufs=4) as pey,
            tc.tile_pool(name="pst", bufs=8) as pst,
            tc.tile_pool(name="psE", bufs=6, space="PSUM") as psE,
        ):
            g_b = pec.tile([P, D], F32)
            b_b = pec.tile([P, D], F32)
            eps_t = pec.tile([P, 1], F32)
            nc.sync.dma_start(g_b, gv[:].partition_broadcast(P))
            nc.sync.dma_start(b_b, bv2[:].partition_broadcast(P))
            nc.vector.memset(eps_t, 1e-5)
            for rt in range(NRT_O):
                qr = peq.tile([P, D], BF16)
                nc.gpsimd.dma_start(qr, qres[rt * P : (rt + 1) * P, :])
                y = pey.tile([P, D], F32)
                for o2 in range(2):
                    ps = psE.tile([P, 512], F32)
                    for hp in range(NOT):
                        nc.tensor.matmul(
                            ps,
                            XO[hp][:, :, rt * P : (rt + 1) * P],
                            wo[:, hp, :, o2 * 512 : (o2 + 1) * 512],
                            start=(hp == 0),
                            stop=(hp == NOT - 1),
                            perf_mode=DR,
                        )
                    nc.vector.tensor_scalar(
                        y[:, o2 * 512 : (o2 + 1) * 512],
                        ps,
                        1.0 / 2048.0,
                        None,
                        op0=ALU.mult,
                    )
                    nc.gpsimd.tensor_tensor(
                        y[:, o2 * 512 : (o2 + 1) * 512],
                        y[:, o2 * 512 : (o2 + 1) * 512],
                        qr[:, o2 * 512 : (o2 + 1) * 512],
                        op=ALU.add,
                    )
                stats = pst.tile([P, 2, 6], F32)
                for sg in range(2):
                    nc.vector.bn_stats(
                        stats[:, sg, :], y[:, sg * 512 : (sg + 1) * 512]
                    )
                mv = pst.tile([P, 2], F32)
                nc.vector.bn_aggr(mv, stats)
                std = pst.tile([P, 1], F32)
                nc.scalar.activation(std, mv[:, 1:2], AF.Sqrt, bias=eps_t)
                rstd = pst.tile([P, 1], F32)
                nc.vector.reciprocal(rstd, std)
                nc.vector.tensor_scalar(
                    y,
                    y,
                    mv[:, 0:1],
                    rstd,
                    op0=ALU.subtract,
                    op1=ALU.mult,
                )
                eng = nc.vector if rt % 2 == 0 else nc.gpsimd
                eng.tensor_mul(y, y, g_b)
                eng.tensor_add(y, y, b_b)
                nc.sync.dma_start(out[rt * P : (rt + 1) * P, :], y)
        pwo_cm.__exit__(None, None, None)
        pxo_cm.__exit__(None, None, None)
        loop_cm.__exit__(None, None, None)
    _split_sync_waits(nc)
    return nc


_NC = None


def _get_nc():
    global _NC
    if _NC is None:
        _NC = build_nc()
    return _NC


def _split_dk(a):
    """[D, N] -> [64, NDT, 2, N] with d = dt*128 + i*64 + p."""
    Dd, N = a.shape
    return np.ascontiguousarray(
        a.reshape(NDT, 2, DK, N).transpose(2, 0, 1, 3)
    )


def prepare_in_maps(q, k, v, Wq, bq, Wk, bk, Wv, bv, Wo, bo, ln_g, ln_b):
    f = np.float32
    f8 = ml_dtypes.float8_e4m3
    q = np.asarray(q, f)
    k = np.asarray(k, f)
    v = np.asarray(v, f)
    wq8 = _split_dk(np.asarray(Wq, f).T * 32.0).astype(f8)
    wk8 = _split_dk(np.asarray(Wk, f).T * 32.0).astype(f8)
    wv8 = _split_dk(np.asarray(Wv, f).T * 32.0).astype(f8)
    wo8 = _split_dk(np.asarray(Wo, f).T * 32.0).astype(f8)
    common = {
        "wq8": wq8,
        "wk8": wk8,
        "wv8": wv8,
        "wo8": wo8,
        "bq32": np.ascontiguousarray((np.asarray(bq, f) * 32.0).reshape(NOT, P).T),
        "bk32": np.ascontiguousarray((np.asarray(bk, f) * 32.0).reshape(NOT, P).T),
        "bvb": (np.asarray(bv, f) * 32.0).astype(ml_dtypes.bfloat16),
        "ln_g": np.asarray(ln_g, f),
        "ln_b": np.asarray(ln_b, f),
        "onesv": np.ones((P, NRT_V * H), ml_dtypes.bfloat16),
        "ones2": np.full(DK, 2.0, np.float32),
    }
    in_maps = []
    for c in range(8):
        b_, half = divmod(c, 2)
        qs = q[b_, half * M : (half + 1) * M, :]
        qres_c = (qs + np.asarray(bo, f)[None, :]).astype(ml_dtypes.bfloat16)
        in_maps.append(
            dict(
                common,
                xq8=_split_dk(qs.T).astype(f8),
                xk8=_split_dk(k[b_].T).astype(f8),
                xv8=_split_dk(v[b_].T).astype(f8),
                qres=np.ascontiguousarray(qres_c),
            )
        )
    return in_maps


def kernel(q, k, v, Wq, bq, Wk, bk, Wv, bv, Wo, bo, ln_g, ln_b):
    nc = _get_nc()
    in_maps = prepare_in_maps(q, k, v, Wq, bq, Wk, bk, Wv, bv, Wo, bo, ln_g, ln_b)
    res = run_bass_kernel_spmd(nc, in_maps, core_ids=list(range(8)))
    out = np.empty((B, S, D), np.float32)
    for c in range(8):
        b_, half = divmod(c, 2)
        out[b_, half * M : (half + 1) * M, :] = res.results[c]["out"]
    return out


# revision 21
# speedup vs baseline: 2.4231x; 1.1246x over previous
"""MultiHeadedAttention block (B=4, S=2048, D=1024, H=16) on 8 TRN2 cores.

Sharding: core c handles batch b=c//2 and query-row half c%2 (1024 rows).
Each core computes full K/V projections for its batch (2x redundant within a
batch pair), attention for all 16 heads over its 1024 query rows, then
O-projection + residual + LayerNorm. No collectives.

All four projections and the QK^T scores run in fp8e4m3 with the DoubleRow
perf mode (2x PE rate). Scale management: weights are stored as W.T*32 in
fp8, activations x in fp8, so Q'=K'=32(xW+b) (stored fp8 in a split-dk
[32p, 2, .] layout for DoubleRow scores), V'=32(xWv+bv) (bf16). Raw scores
are 1024*(QK^T); the softmax exp applies scale 1/8192 = 1/(1024*sqrt(64)).
The ones-column of V gives the softmax denominator D via the PV matmul; the
reciprocal is broadcast with a ones(=2.0) stationary vector so
xo = pv * (2/D) = 64*(attn_out + bv) in fp8, and phase E folds the
1/(64*32) into a 1/2048 multiply before the residual add + LayerNorm.

The softmax exp is split across three engines: exact Exp on the Activation
engine plus the int16-bitcast approximation exp(x) ~= bitcast_bf16(
int16(x*128*log2e + 127*128 - 5.5)) on Pool and DVE (max ~3% weight error,
negligible through the diffuse softmax at this tolerance).
"""

import sys

if "/opt/trn_rl_repo" not in sys.path:
    sys.path.insert(0, "/opt/trn_rl_repo")

import ml_dtypes
import numpy as np

import concourse.bass as bass
import concourse.mybir as mybir
import concourse.tile as tile
from concourse.bass_utils import run_bass_kernel_spmd

B, S, D, H, DK = 4, 2048, 1024, 16, 64
P = 128
M = S // 2          # query rows per core
NDT = D // P        # 8 contraction chunks of 128
NOT = D // P        # 8 output-feature chunks (= head pairs)
NHP = H // 2        # 8 head pairs
NKT = S // P        # 16 key chunks of 128
NQT = M // 512      # 2 query 512-chunks
NRT_K = S // 512    # 4 key-row 512-chunks
NRT_V = S // P      # 16 V row chunks
NRT_O = M // P      # 8 output row chunks
KG = 2              # k-chunks per exp group
NKG = NKT // KG     # 8 exp groups per (head, qt)
F32 = mybir.dt.float32
F8 = mybir.dt.float8e4
BF16 = mybir.dt.bfloat16
I16 = mybir.dt.int16
MM_DT = mybir.dt.float32r
AF = mybir.ActivationFunctionType
ALU = mybir.AluOpType
DR = mybir.MatmulPerfMode.DoubleRow

LOG2E = 1.4426950408889634
EXP_SCALE = 1.0 / 8192.0                 # 1/(32*32*sqrt(DK))
EXP_A = 128.0 * LOG2E * EXP_SCALE        # int16-bitcast exp multiplier
EXP_B = 127.0 * 128.0 - 5.5              # exponent bias - mean sawtooth corr
# exp engine rotation per (head, qt): Pool cannot read PSUM, so split
# between Activation (exact) and DVE (bitcast approx) only
EXP_ENGINES = "AADAAADA"


def _split_sync_waits(nc, max_waits=1):
    """Split instructions carrying more than max_waits sem waits.

    The container's walrus rejects instructions with multiple sync wait
    commands, so excess waits move onto NoOp instructions inserted just
    before, on the same engine.
    """
    idx = 0
    for f in nc.m.functions:
        for blk in f.blocks:
            newl = []
            for inst in blk.instructions:
                si = inst.sync_info
                waits = list(si.on_wait) if si is not None and si.on_wait else []
                if len(waits) > max_waits:
                    extra = waits[max_waits:]
                    si.on_wait = waits[:max_waits]
                    for j in range(0, len(extra), max_waits):
                        nop = mybir.InstNoOp(name=f"I-wsplit-{idx}", ins=[], outs=[])
                        idx += 1
                        nop.engine = inst.engine
                        nop.sync_info = mybir.SyncInfo(
                            on_wait=extra[j : j + max_waits], on_update=[]
                        )
                        newl.append(nop)
                newl.append(inst)
            blk.instructions = newl


def build_nc(loops=0):
    nc = bass.Bass()
    xq8 = nc.dram_tensor("xq8", [DK, NDT, 2, M], F8, kind="ExternalInput")
    xk8 = nc.dram_tensor("xk8", [DK, NDT, 2, S], F8, kind="ExternalInput")
    xv8 = nc.dram_tensor("xv8", [DK, NDT, 2, S], F8, kind="ExternalInput")
    wq8 = nc.dram_tensor("wq8", [DK, NDT, 2, D], F8, kind="ExternalInput")
    wk8 = nc.dram_tensor("wk8", [DK, NDT, 2, D], F8, kind="ExternalInput")
    wv8 = nc.dram_tensor("wv8", [DK, NDT, 2, D], F8, kind="ExternalInput")
    wo8 = nc.dram_tensor("wo8", [DK, NDT, 2, D], F8, kind="ExternalInput")
    bq32 = nc.dram_tensor("bq32", [P, NOT], F32, kind="ExternalInput")
    bk32 = nc.dram_tensor("bk32", [P, NOT], F32, kind="ExternalInput")
    bvb = nc.dram_tensor("bvb", [D], BF16, kind="ExternalInput")
    qres = nc.dram_tensor("qres", [M, D], BF16, kind="ExternalInput")
    gv = nc.dram_tensor("ln_g", [D], F32, kind="ExternalInput")
    bv2 = nc.dram_tensor("ln_b", [D], F32, kind="ExternalInput")
    out = nc.dram_tensor("out", [M, D], F32, kind="ExternalOutput")

    import contextlib

    with tile.TileContext(nc) as tc:
        loop_cm = tc.For_i(0, loops, 1) if loops else contextlib.nullcontext()
        loop_cm.__enter__()
        pxo_cm = tc.tile_pool(name="pxo", bufs=1)
        pxo = pxo_cm.__enter__()
        with (
            tc.tile_pool(name="pqv", bufs=1) as pqv,
        ):
            # attention outputs, fp8 split-head-pair layout for O-proj DR
            XO = [
                pxo.tile([DK, 2, M], F8, tag=f"XO{i}", name=f"XO{i}")
                for i in range(NHP)
            ]

            # Q' fp8, 2 heads across partitions: head h at partition
            # (h%2)*64, slot h//2, sub i = dk//32
            QT = pqv.tile([P, 8, 2, M], F8, tag="QT", name="QT")
            bq_p = pqv.tile([P, NOT], F32)
            bk_p = pqv.tile([P, NOT], F32)
            bv_b = pqv.tile([P, D], BF16)

            # Vt: [P, 2, H, DK] fp8; sub 1 is a 0.5-constant block so the
            # PV matmul replicates den/2 across output partitions 64..127
            Vt = []
            for rt in range(NRT_V):
                t = pqv.tile([P, H, 2, DK], F8, tag=f"Vt{rt}", name=f"Vt{rt}")
                nc.gpsimd.memset(t[:, :, 1, :], 0.5)
                Vt.append(t)

            # wv/xv load early so phase B starts without a DMA stall
            pwv_cm = tc.tile_pool(name="pwv", bufs=NDT, side="right")
            pwv = pwv_cm.__enter__()
            wv = [
                pwv.tile([DK, 2, D], F8, tag="wv", name=f"wv{dt}")
                for dt in range(NDT)
            ]

            pbx_cm = tc.tile_pool(name="pbx", bufs=NDT, side="right")
            pbx = pbx_cm.__enter__()
            xv = [
                pbx.tile([DK, 2, S], F8, tag="xv", name=f"xv{dt}")
                for dt in range(NDT)
            ]

            psAB_cm = tc.tile_pool(name="psAB", bufs=4, space="PSUM")
            psAB = psAB_cm.__enter__()
            psg_cm = tc.tile_pool(name="psg", bufs=3)
            psg = psg_cm.__enter__()

            # ---- Phase A: Q' = 32*(Wq @ x_q^T) + 32 bq, fp8 DR
            with (
                tc.tile_pool(name="pa", bufs=NDT) as pa,
            ):
                wq = []
                xq = []
                for dt in range(NDT):
                    wt = pa.tile([DK, 2, D], F8, tag="wq", name=f"wq{dt}")
                    nc.sync.dma_start(wt, wq8[:, dt, :, :])
                    wq.append(wt)
                    xt = pa.tile([DK, 2, M], F8, tag="xq", name=f"xq{dt}")
                    nc.sync.dma_start(xt, xq8[:, dt, :, :])
                    xq.append(xt)
                # behind phase A's own loads in the HWDGE queue: biases,
                # then the wv/xv pairs phase B consumes in dt order
                nc.sync.dma_start(bq_p, bq32[:, :])
                nc.sync.dma_start(bk_p, bk32[:, :])
                nc.sync.dma_start(bv_b, bvb[:].partition_broadcast(P))
                for dt in range(NDT):
                    nc.sync.dma_start(wv[dt], wv8[:, dt, :, :])
                    nc.sync.dma_start(xv[dt], xv8[:, dt, :, :])
                for ot in range(NOT):
                    stage = psg.tile([P, M], F8, tag="qstg", name="qstg")
                    for qt in range(NQT):
                        ps = psAB.tile([P, 512], F32, tag="ps", name="ps")
                        for dt in range(NDT):
                            nc.tensor.matmul(
                                ps,
                                wq[dt][:, :, ot * P : (ot + 1) * P],
                                xq[dt][:, :, qt * 512 : (qt + 1) * 512],
                                start=(dt == 0),
                                stop=(dt == NDT - 1),
                                perf_mode=DR,
                            )
                        nc.vector.tensor_scalar_add(
                            stage[:, qt * 512 : (qt + 1) * 512],
                            ps,
                            bq_p[:, ot : ot + 1],
                        )
                    # repack the whole ot row into the split-dk
                    # 2-heads-across-partitions layout via 4 DMAs
                    for h01 in range(2):
                        h = 2 * ot + h01
                        for i in range(2):
                            pb = h01 * DK + i * 32
                            deng = nc.sync if i == 0 else nc.gpsimd
                            deng.dma_start(
                                QT[(h % 2) * 64 : (h % 2) * 64 + 32, h // 2, i, :],
                                stage[pb : pb + 32, :],
                            )

            # xk/wk load during phase B so phase D starts without a DMA stall
            pdx_cm = tc.tile_pool(name="pdx", bufs=NDT)
            pdx = pdx_cm.__enter__()
            xk = []
            wk = []
            for dt in range(NDT):
                xt = pdx.tile([DK, 2, S], F8, tag="xk", name=f"xk{dt}")
                nc.sync.dma_start(xt, xk8[:, dt, :, :])
                xk.append(xt)
                wt = pdx.tile([DK, 2, D], F8, tag="wk", name=f"wk{dt}")
                nc.sync.dma_start(wt, wk8[:, dt, :, :])
                wk.append(wt)

            # ---- Phase B: V' = 32*(x_v @ Wv^T + bv), fp8 DR, bf16 out
            for rt in range(NRT_V):
                for o2 in range(2):
                    ps = psAB.tile([P, 512], F32, tag="ps", name="ps")
                    for dt in range(NDT):
                        nc.tensor.matmul(
                            ps,
                            xv[dt][:, :, rt * P : (rt + 1) * P],
                            wv[dt][:, :, o2 * 512 : (o2 + 1) * 512],
                            start=(dt == 0),
                            stop=(dt == NDT - 1),
                            perf_mode=DR,
                        )
                    nc.vector.tensor_tensor(
                        Vt[rt][:, o2 * 8 : (o2 + 1) * 8, 0, :],
                        ps[:, :].rearrange("p (h e) -> p h e", e=DK),
                        bv_b[:, o2 * 512 : (o2 + 1) * 512].rearrange(
                            "p (h e) -> p h e", e=DK
                        ),
                        op=ALU.add,
                    )

            pbx_cm.__exit__(None, None, None)
            pwv_cm.__exit__(None, None, None)
            psAB_cm.__exit__(None, None, None)

            # wo prefetch during D so phase E starts without a DMA stall
            pwo_cm = tc.tile_pool(name="pwo", bufs=NDT, side="right")
            pwo = pwo_cm.__enter__()
            wo = []
            for dt in range(NDT):
                t = pwo.tile([DK, 2, D], F8, tag="wo", name=f"wo{dt}")
                nc.sync.dma_start(t, wo8[:, dt, :, :])
                wo.append(t)

            # ---- Phase D: K' projection fused with attention
            with (
                tc.tile_pool(name="pdkt", bufs=1) as pdkt,
                tc.tile_pool(name="pde", bufs=4) as pde,
                tc.tile_pool(name="pdr", bufs=4) as pdr,
                tc.tile_pool(name="psS", bufs=3, space="PSUM") as psS,
                tc.tile_pool(name="psPV", bufs=2, space="PSUM") as psPV,
            ):
                # K' fp8, same 2-heads-across-partitions layout as QT
                KT = pdkt.tile([P, 8, 2, S], F8, tag="KT", name="KT")

                def kproj(hp):
                    stage = psg.tile([P, S], F8, tag="kstg", name="kstg")
                    for rt in range(NRT_K):
                        ps = psS.tile([P, KG, 512], F32, tag="ss", name="ss")[
                            :, 0, :
                        ]
                        for dt in range(NDT):
                            nc.tensor.matmul(
                                ps,
                                wk[dt][:, :, hp * P : (hp + 1) * P],
                                xk[dt][:, :, rt * 512 : (rt + 1) * 512],
                                start=(dt == 0),
                                stop=(dt == NDT - 1),
                                perf_mode=DR,
                            )
                        nc.vector.tensor_scalar_add(
                            stage[:, rt * 512 : (rt + 1) * 512],
                            ps,
                            bk_p[:, hp : hp + 1],
                        )
                    for h01 in range(2):
                        h = 2 * hp + h01
                        for i in range(2):
                            pb = h01 * DK + i * 32
                            deng = nc.sync if i == 0 else nc.gpsimd
                            deng.dma_start(
                                KT[(h % 2) * 64 : (h % 2) * 64 + 32, h // 2, i, :],
                                stage[pb : pb + 32, :],
                            )

                def attn2(hp):
                    """Both heads of a pair per key-chunk: one score tile
                    [128, 2(head), 512] -> one exp op -> two pv matmuls.
                    Normalization runs off the PE: reciprocal on DVE, a
                    DMA partition-broadcast, and the multiply on Pool."""
                    xo_t = XO[hp]
                    for qt in range(NQT):
                        pvs_ = [
                            psPV.tile([2 * DK, 512], F32, tag="pv", name="pv")
                            for _ in range(2)
                        ]
                        for kt in range(NKT):
                            ss = psS.tile([P, KG, 512], F32, tag="ss", name="ss")
                            for h01 in range(2):
                                kb = h01 * 64
                                nc.tensor.matmul(
                                    ss[:, h01, :],
                                    KT[kb : kb + 32, hp, :, kt * P : (kt + 1) * P],
                                    QT[
                                        kb : kb + 32,
                                        hp,
                                        :,
                                        qt * 512 : (qt + 1) * 512,
                                    ],
                                    start=True,
                                    stop=True,
                                    perf_mode=DR,
                                )
                            ex = pde.tile([P, KG, 512], I16, tag="ex", name="ex")
                            if kt % 8 in (2, 5, 7):
                                nc.vector.tensor_scalar(
                                    ex,
                                    ss,
                                    EXP_A,
                                    EXP_B,
                                    op0=ALU.mult,
                                    op1=ALU.add,
                                )
                            else:
                                nc.scalar.activation(
                                    ex[:, :, :].bitcast(BF16),
                                    ss,
                                    AF.Exp,
                                    scale=EXP_SCALE,
                                )
                            for h01 in range(2):
                                nc.tensor.matmul(
                                    pvs_[h01],
                                    Vt[kt][:, 2 * hp + h01, :, :],
                                    ex[:, h01, :].bitcast(BF16),
                                    start=(kt == 0),
                                    stop=(kt == NKT - 1),
                                )
                        for h01 in range(2):
                            rc64 = pdr.tile([DK, 512], F32, tag="rc", name="rc")
                            nc.vector.reciprocal(
                                rc64, pvs_[h01][DK : 2 * DK, :]
                            )
                            nc.vector.tensor_tensor(
                                xo_t[:, h01, qt * 512 : (qt + 1) * 512],
                                pvs_[h01][0:DK, :],
                                rc64,
                                op=ALU.mult,
                            )

                kproj(0)
                for hp in range(NHP):
                    if hp + 1 < NHP:
                        kproj(hp + 1)
                    attn2(hp)

            pdx_cm.__exit__(None, None, None)
            psg_cm.__exit__(None, None, None)

        # ---- Phase E: out = LN(x_o @ Wo^T + bo + q)  (bo pre-added to qres)
        with (
            tc.tile_pool(name="pec", bufs=1) as pec,
            tc.tile_pool(name="peq", bufs=4) as peq,
            tc.tile_pool(name="pey", bufs=4) as pey,
            tc.tile_pool(name="pst", bufs=8) as pst,
            tc.tile_pool(name="psE", bufs=6, space="PSUM") as psE,
        ):
            g_b = pec.tile([P, D], F32)
            b_b = pec.tile([P, D], F32)
            eps_t = pec.tile([P, 1], F32)
            nc.sync.dma_start(g_b, gv[:].partition_broadcast(P))
            nc.sync.dma_start(b_b, bv2[:].partition_broadcast(P))
            nc.vector.memset(eps_t, 1e-5)
            for rt in range(NRT_O):
                qr = peq.tile([P, D], BF16)
                nc.sync.dma_start(qr, qres[rt * P : (rt + 1) * P, :])
                y = pey.tile([P, D], F32)
                for o2 in range(2):
                    ps = psE.tile([P, 512], F32)
                    for hp in range(NOT):
                        nc.tensor.matmul(
                            ps,
                            XO[hp][:, :, rt * P : (rt + 1) * P],
                            wo[hp][:, :, o2 * 512 : (o2 + 1) * 512],
                            start=(hp == 0),
                            stop=(hp == NOT - 1),
                            perf_mode=DR,
                        )
                    nc.scalar.activation(
                        y[:, o2 * 512 : (o2 + 1) * 512],
                        ps,
                        AF.Copy,
                        scale=1.0 / 2048.0,
                    )
                    nc.gpsimd.tensor_tensor(
                        y[:, o2 * 512 : (o2 + 1) * 512],
                        y[:, o2 * 512 : (o2 + 1) * 512],
                        qr[:, o2 * 512 : (o2 + 1) * 512],
                        op=ALU.add,
                    )
                stats = pst.tile([P, 2, 6], F32)
                for sg in range(2):
                    nc.vector.bn_stats(
                        stats[:, sg, :], y[:, sg * 512 : (sg + 1) * 512]
                    )
                mv = pst.tile([P, 2], F32)
                nc.vector.bn_aggr(mv, stats)
                std = pst.tile([P, 1], F32)
                nc.scalar.activation(std, mv[:, 1:2], AF.Sqrt, bias=eps_t)
                rstd = pst.tile([P, 1], F32)
                nc.vector.reciprocal(rstd, std)
                nc.vector.tensor_scalar(
                    y,
                    y,
                    mv[:, 0:1],
                    rstd,
                    op0=ALU.subtract,
                    op1=ALU.mult,
                )
                eng = nc.vector if rt % 2 == 0 else nc.gpsimd
                eng.tensor_mul(y, y, g_b)
                eng.tensor_add(y, y, b_b)
                nc.sync.dma_start(out[rt * P : (rt + 1) * P, :], y)
        pwo_cm.__exit__(None, None, None)
        pxo_cm.__exit__(None, None, None)
        loop_cm.__exit__(None, None, None)
    _split_sync_waits(nc)
    return nc


_NC = None


def _get_nc():
    global _NC
    if _NC is None:
        _NC = build_nc()
    return _NC


def _split_dk(a):
    """[D, N] -> [64, NDT, 2, N] with d = dt*128 + i*64 + p."""
    Dd, N = a.shape
    return np.ascontiguousarray(
        a.reshape(NDT, 2, DK, N).transpose(2, 0, 1, 3)
    )


def prepare_in_maps(q, k, v, Wq, bq, Wk, bk, Wv, bv, Wo, bo, ln_g, ln_b):
    f = np.float32
    f8 = ml_dtypes.float8_e4m3
    q = np.asarray(q, f)
    k = np.asarray(k, f)
    v = np.asarray(v, f)
    wq8 = _split_dk(np.asarray(Wq, f).T * 32.0).astype(f8)
    wk8 = _split_dk(np.asarray(Wk, f).T * 32.0).astype(f8)
    wv8 = _split_dk(np.asarray(Wv, f).T * 32.0).astype(f8)
    wo8 = _split_dk(np.asarray(Wo, f).T * 32.0).astype(f8)
    common = {
        "wq8": wq8,
        "wk8": wk8,
        "wv8": wv8,
        "wo8": wo8,
        "bq32": np.ascontiguousarray((np.asarray(bq, f) * 32.0).reshape(NOT, P).T),
        "bk32": np.ascontiguousarray((np.asarray(bk, f) * 32.0).reshape(NOT, P).T),
        "bvb": (np.asarray(bv, f) * 32.0).astype(ml_dtypes.bfloat16),
        "ln_g": np.asarray(ln_g, f),
        "ln_b": np.asarray(ln_b, f),
    }
    in_maps = []
    for c in range(8):
        b_, half = divmod(c, 2)
        qs = q[b_, half * M : (half + 1) * M, :]
        qres_c = (qs + np.asarray(bo, f)[None, :]).astype(ml_dtypes.bfloat16)
        in_maps.append(
            dict(
                common,
                xq8=_split_dk(qs.T).astype(f8),
                xk8=_split_dk(k[b_].T).astype(f8),
                xv8=_split_dk(v[b_].T).astype(f8),
                qres=np.ascontiguousarray(qres_c),
            )
        )
    return in_maps


def kernel(q, k, v, Wq, bq, Wk, bk, Wv, bv, Wo, bo, ln_g, ln_b):
    nc = _get_nc()
    in_maps = prepare_in_maps(q, k, v, Wq, bq, Wk, bk, Wv, bv, Wo, bo, ln_g, ln_b)
    res = run_bass_kernel_spmd(nc, in_maps, core_ids=list(range(8)))
    out = np.empty((B, S, D), np.float32)
    for c in range(8):
        b_, half = divmod(c, 2)
        out[b_, half * M : (half + 1) * M, :] = res.results[c]["out"]
    return out


# revision 36
# speedup vs baseline: 4.7159x; 1.9462x over previous
"""MultiHeadedAttention block (B=4, S=2048, D=1024, H=16) on 8 TRN2 cores.

Sharding: core c handles batch b=c//2 and query-row half c%2 (1024 rows).
Each core computes full K/V projections for its batch (2x redundant within a
batch pair), attention for all 16 heads over its 1024 query rows, then
O-projection + residual + LayerNorm. No collectives.

All four projections and the QK^T scores run in fp8e4m3 with the DoubleRow
perf mode (2x PE rate). Scale management: weights are stored as W.T*32 in
fp8, activations x in fp8, so Q'=K'=32(xW+b) (stored fp8 in a split-dk
[32p, 2, .] layout for DoubleRow scores), V'=32(xWv+bv) (bf16). Raw scores
are 1024*(QK^T); the softmax exp applies scale 1/8192 = 1/(1024*sqrt(64)).
The ones-column of V gives the softmax denominator D via the PV matmul; the
reciprocal is broadcast with a ones(=2.0) stationary vector so
xo = pv * (2/D) = 64*(attn_out + bv) in fp8, and phase E folds the
1/(64*32) into a 1/2048 multiply before the residual add + LayerNorm.

The softmax exp is split across three engines: exact Exp on the Activation
engine plus the int16-bitcast approximation exp(x) ~= bitcast_bf16(
int16(x*128*log2e + 127*128 - 5.5)) on Pool and DVE (max ~3% weight error,
negligible through the diffuse softmax at this tolerance).
"""

import sys

if "/opt/trn_rl_repo" not in sys.path:
    sys.path.insert(0, "/opt/trn_rl_repo")

import ml_dtypes
import numpy as np

import concourse.bass as bass
import concourse.mybir as mybir
import concourse.tile as tile
from concourse.bass_utils import run_bass_kernel_spmd

B, S, D, H, DK = 4, 2048, 1024, 16, 64
P = 128
M = S // 2          # query rows per core
NDT = D // P        # 8 contraction chunks of 128
NOT = D // P        # 8 output-feature chunks (= head pairs)
NHP = H // 2        # 8 head pairs
NKT = S // P        # 16 key chunks of 128
NQT = M // 512      # 2 query 512-chunks
NRT_K = S // 512    # 4 key-row 512-chunks
NRT_V = S // P      # 16 V row chunks
NRT_O = M // P      # 8 output row chunks
KG = 2              # k-chunks per exp group
NKG = NKT // KG     # 8 exp groups per (head, qt)
F32 = mybir.dt.float32
F8 = mybir.dt.float8e4
BF16 = mybir.dt.bfloat16
I16 = mybir.dt.int16
MM_DT = mybir.dt.float32r
AF = mybir.ActivationFunctionType
ALU = mybir.AluOpType
DR = mybir.MatmulPerfMode.DoubleRow

LOG2E = 1.4426950408889634
EXP_SCALE = 1.0 / 8192.0                 # 1/(32*32*sqrt(DK))
EXP_A = 128.0 * LOG2E * EXP_SCALE        # int16-bitcast exp multiplier
EXP_B = 127.0 * 128.0 - 5.5              # exponent bias - mean sawtooth corr
# exp engine rotation per (head, qt): Pool cannot read PSUM, so split
# between Activation (exact) and DVE (bitcast approx) only
EXP_ENGINES = "AADAAADA"


def _split_sync_waits(nc, max_waits=1):
    """Split instructions carrying more than max_waits sem waits.

    The container's walrus rejects instructions with multiple sync wait
    commands, so excess waits move onto NoOp instructions inserted just
    before, on the same engine.
    """
    idx = 0
    for f in nc.m.functions:
        for blk in f.blocks:
            newl = []
            for inst in blk.instructions:
                si = inst.sync_info
                waits = list(si.on_wait) if si is not None and si.on_wait else []
                if len(waits) > max_waits:
                    extra = waits[max_waits:]
                    si.on_wait = waits[:max_waits]
                    for j in range(0, len(extra), max_waits):
                        nop = mybir.InstNoOp(name=f"I-wsplit-{idx}", ins=[], outs=[])
                        idx += 1
                        nop.engine = inst.engine
                        nop.sync_info = mybir.SyncInfo(
                            on_wait=extra[j : j + max_waits], on_update=[]
                        )
                        newl.append(nop)
                newl.append(inst)
            blk.instructions = newl


def build_nc(loops=0):
    nc = bass.Bass()
    xq8 = nc.dram_tensor("xq8", [DK, NDT, 2, M], F8, kind="ExternalInput")
    xk8 = nc.dram_tensor("xk8", [DK, NDT, 2, S], F8, kind="ExternalInput")
    xv8 = nc.dram_tensor("xv8", [DK, NDT, 2, S], F8, kind="ExternalInput")
    wq8 = nc.dram_tensor("wq8", [DK, NDT, 2, D], F8, kind="ExternalInput")
    wk8 = nc.dram_tensor("wk8", [DK, NDT, 2, D], F8, kind="ExternalInput")
    wv8 = nc.dram_tensor("wv8", [DK, NDT, 2, D], F8, kind="ExternalInput")
    wo8 = nc.dram_tensor("wo8", [DK, NDT, 2, D], F8, kind="ExternalInput")
    bq32 = nc.dram_tensor("bq32", [P, NOT], F32, kind="ExternalInput")
    bk32 = nc.dram_tensor("bk32", [P, NOT], F32, kind="ExternalInput")
    bkr = nc.dram_tensor("bkr", [NHP, P], BF16, kind="ExternalInput")
    bvb = nc.dram_tensor("bvb", [D], BF16, kind="ExternalInput")
    qres = nc.dram_tensor("qres", [M, D], BF16, kind="ExternalInput")
    gv = nc.dram_tensor("ln_g", [D], F32, kind="ExternalInput")
    bv2 = nc.dram_tensor("ln_b", [D], F32, kind="ExternalInput")
    out = nc.dram_tensor("out", [M, D], F32, kind="ExternalOutput")

    import contextlib

    with tile.TileContext(nc) as tc:
        loop_cm = tc.For_i(0, loops, 1) if loops else contextlib.nullcontext()
        loop_cm.__enter__()
        pxo_cm = tc.tile_pool(name="pxo", bufs=1)
        pxo = pxo_cm.__enter__()
        with (
            tc.tile_pool(name="pqv", bufs=1) as pqv,
        ):
            # attention outputs, fp8 split-head-pair layout for O-proj DR
            XO = [
                pxo.tile([DK, 2, M], F8, tag=f"XO{i}", name=f"XO{i}")
                for i in range(NHP)
            ]

            # Q' fp8, 2 heads across partitions: head h at partition
            # (h%2)*64, slot h//2, sub i = dk//32
            QT = pqv.tile([P, 8, 2, M], F8, tag="QT", name="QT")
            bq_p = pqv.tile([P, NOT], F32)
            bkr_t = pqv.tile([1, NHP, P], BF16)
            ones_bf = pqv.tile([1, 512], BF16)
            nc.vector.memset(ones_bf, 1.0)
            nc.sync.dma_start(bkr_t, bkr[:, :].rearrange("a p -> (a p)").partition_broadcast(1))
            bk_p = pqv.tile([P, NOT], F32)
            bv_b = pqv.tile([P, D], BF16)

            # Vt: [P, 2, H, DK] fp8; sub 1 is a 0.5-constant block so the
            # PV matmul replicates den/2 across output partitions 64..127
            Vt = []
            for rt in range(NRT_V):
                t = pqv.tile([P, H, 2, DK], F8, tag=f"Vt{rt}", name=f"Vt{rt}")
                nc.vector.memset(t[:, :, 1, :], 0.5)
                Vt.append(t)

            # wv/xv load early so phase B starts without a DMA stall
            pwv_cm = tc.tile_pool(name="pwv", bufs=NDT // 2, side="right")
            pwv = pwv_cm.__enter__()
            wvp = [
                pwv.tile([DK, 2, 2, D], F8, tag="wv", name=f"wv{dp}")
                for dp in range(NDT // 2)
            ]
            wv = [wvp[dt // 2][:, dt % 2] for dt in range(NDT)]

            pbx_cm = tc.tile_pool(name="pbx", bufs=NDT // 2, side="right")
            pbx = pbx_cm.__enter__()
            xvp = [
                pbx.tile([DK, 2, 2, S], F8, tag="xv", name=f"xv{dp}")
                for dp in range(NDT // 2)
            ]
            xv = [xvp[dt // 2][:, dt % 2] for dt in range(NDT)]

            psAB_cm = tc.tile_pool(name="psAB", bufs=4, space="PSUM")
            psAB = psAB_cm.__enter__()
            psg_cm = tc.tile_pool(name="psg", bufs=3)
            psg = psg_cm.__enter__()

            # ---- Phase A: Q' = 32*(Wq @ x_q^T) + 32 bq, fp8 DR
            with (
                tc.tile_pool(name="pa", bufs=NDT // 2) as pa,
            ):
                wqp = []
                xqp = []
                for dp in range(NDT // 2):
                    wt = pa.tile([DK, 2, 2, D], F8, tag="wq", name=f"wq{dp}")
                    nc.sync.dma_start(wt, wq8[:, 2 * dp : 2 * dp + 2, :, :])
                    wqp.append(wt)
                    xt = pa.tile([DK, 2, 2, M], F8, tag="xq", name=f"xq{dp}")
                    nc.sync.dma_start(xt, xq8[:, 2 * dp : 2 * dp + 2, :, :])
                    xqp.append(xt)
                wq = [wqp[dt // 2][:, dt % 2] for dt in range(NDT)]
                xq = [xqp[dt // 2][:, dt % 2] for dt in range(NDT)]
                # behind phase A's own loads in the HWDGE queue: biases,
                # then the wv/xv pairs phase B consumes in dt order
                nc.sync.dma_start(bq_p, bq32[:, :])
                nc.sync.dma_start(bk_p, bk32[:, :])
                nc.sync.dma_start(bv_b, bvb[:].partition_broadcast(P))
                for dp in range(NDT // 2):
                    nc.sync.dma_start(wvp[dp], wv8[:, 2 * dp : 2 * dp + 2, :, :])
                    nc.sync.dma_start(xvp[dp], xv8[:, 2 * dp : 2 * dp + 2, :, :])
                for ot in range(NOT):
                    stage = psg.tile([P, M], F8, tag="qstg", name="qstg")
                    for qt in range(NQT):
                        ps = psAB.tile([P, 512], F32, tag="ps", name="ps")
                        for dt in range(NDT):
                            nc.tensor.matmul(
                                ps,
                                wq[dt][:, :, ot * P : (ot + 1) * P],
                                xq[dt][:, :, qt * 512 : (qt + 1) * 512],
                                start=(dt == 0),
                                stop=(dt == NDT - 1),
                                perf_mode=DR,
                            )
                        nc.vector.tensor_scalar_add(
                            stage[:, qt * 512 : (qt + 1) * 512],
                            ps,
                            bq_p[:, ot : ot + 1],
                        )
                    # repack the whole ot row into the split-dk
                    # 2-heads-across-partitions layout via 4 DMAs
                    for h01 in range(2):
                        h = 2 * ot + h01
                        for i in range(2):
                            pb = h01 * DK + i * 32
                            (nc.sync if i == 0 else nc.gpsimd).dma_start(
                                QT[(h % 2) * 64 : (h % 2) * 64 + 32, h // 2, i, :],
                                stage[pb : pb + 32, :],
                            )

            # xk/wk load during phase B so phase D starts without a DMA stall
            pdx_cm = tc.tile_pool(name="pdx", bufs=NDT // 2)
            pdx = pdx_cm.__enter__()
            xkp = []
            wkp = []
            for dp in range(NDT // 2):
                xt = pdx.tile([DK, 2, 2, S], F8, tag="xk", name=f"xk{dp}")
                nc.sync.dma_start(xt, xk8[:, 2 * dp : 2 * dp + 2, :, :])
                xkp.append(xt)
                wt = pdx.tile([DK, 2, 2, D], F8, tag="wk", name=f"wk{dp}")
                nc.sync.dma_start(wt, wk8[:, 2 * dp : 2 * dp + 2, :, :])
                wkp.append(wt)
            xk = [xkp[dt // 2][:, dt % 2] for dt in range(NDT)]
            wk = [wkp[dt // 2][:, dt % 2] for dt in range(NDT)]

            # ---- Phase B: V' = 32*(x_v @ Wv^T + bv), fp8 DR, bf16 out
            for rt in range(NRT_V):
                for o2 in range(2):
                    ps = psAB.tile([P, 512], F32, tag="ps", name="ps")
                    for dt in range(NDT):
                        nc.tensor.matmul(
                            ps,
                            xv[dt][:, :, rt * P : (rt + 1) * P],
                            wv[dt][:, :, o2 * 512 : (o2 + 1) * 512],
                            start=(dt == 0),
                            stop=(dt == NDT - 1),
                            perf_mode=DR,
                        )
                    nc.vector.tensor_tensor(
                        Vt[rt][:, o2 * 8 : (o2 + 1) * 8, 0, :],
                        ps[:, :].rearrange("p (h e) -> p h e", e=DK),
                        bv_b[:, o2 * 512 : (o2 + 1) * 512].rearrange(
                            "p (h e) -> p h e", e=DK
                        ),
                        op=ALU.add,
                    )

            pbx_cm.__exit__(None, None, None)
            pwv_cm.__exit__(None, None, None)
            psAB_cm.__exit__(None, None, None)

            # wo prefetch during D so phase E starts without a DMA stall
            pwo_cm = tc.tile_pool(name="pwo", bufs=NDT, side="right")
            pwo = pwo_cm.__enter__()
            wo = []
            for dt in range(NDT):
                t = pwo.tile([DK, 2, D], F8, tag="wo", name=f"wo{dt}")
                nc.sync.dma_start(t, wo8[:, dt, :, :])
                wo.append(t)
            pec_cm = tc.tile_pool(name="pec", bufs=1, side="right")
            pec = pec_cm.__enter__()
            g_b = pec.tile([P, D], F32)
            b_b = pec.tile([P, D], F32)
            eps_t = pec.tile([P, 1], F32)
            nc.sync.dma_start(g_b, gv[:].partition_broadcast(P))
            nc.sync.dma_start(b_b, bv2[:].partition_broadcast(P))
            nc.vector.memset(eps_t, 1e-5)

            # ---- Phase D: K' projection fused with attention
            with (
                tc.tile_pool(name="pdkt", bufs=1) as pdkt,
                tc.tile_pool(name="pde", bufs=6) as pde,
                tc.tile_pool(name="pdr", bufs=4) as pdr,
                tc.tile_pool(name="psS", bufs=3, space="PSUM") as psS,
                tc.tile_pool(name="psPV", bufs=2, space="PSUM") as psPV,
            ):
                # K' fp8, same 2-heads-across-partitions layout as QT
                KT = pdkt.tile([P, 8, 2, S], F8, tag="KT", name="KT")

                def kproj(hp):
                    stage = psg.tile([P, S], F8, tag="kstg", name="kstg")
                    for rt in range(NRT_K):
                        ps = psS.tile([P, KG, 512], F32, tag="ss", name="ss")[
                            :, 0, :
                        ]
                        for dt in range(NDT):
                            nc.tensor.matmul(
                                ps,
                                wk[dt][:, :, hp * P : (hp + 1) * P],
                                xk[dt][:, :, rt * 512 : (rt + 1) * 512],
                                start=(dt == 0),
                                stop=False,
                                perf_mode=DR,
                            )
                        # bias via a 1-row accumulation matmul so the stage
                        # copy below needs no per-partition bias operand
                        nc.tensor.matmul(
                            ps,
                            bkr_t[:, hp, :],
                            ones_bf,
                            start=False,
                            stop=True,
                        )
                        nc.scalar.activation(
                            stage[:, rt * 512 : (rt + 1) * 512],
                            ps,
                            AF.Copy,
                        )
                    for h01 in range(2):
                        h = 2 * hp + h01
                        for i in range(2):
                            pb = h01 * DK + i * 32
                            (nc.sync if i == 0 else nc.gpsimd).dma_start(
                                KT[(h % 2) * 64 : (h % 2) * 64 + 32, h // 2, i, :],
                                stage[pb : pb + 32, :],
                            )

                def attn2(hp):
                    """Both heads of a pair per key-chunk: one score tile
                    [128, 2(head), 512] -> one exp op -> two pv matmuls.
                    Normalization runs off the PE: reciprocal on DVE, a
                    DMA partition-broadcast, and the multiply on Pool."""
                    xo_t = XO[hp]
                    for qt in range(NQT):
                        pvs_ = [
                            psPV.tile([2 * DK, 512], F32, tag="pv", name="pv")
                            for _ in range(2)
                        ]

                        pending = []

                        def pv_mms(kt, ex):
                            for h01 in range(2):
                                nc.tensor.matmul(
                                    pvs_[h01],
                                    Vt[kt][:, 2 * hp + h01, :, :],
                                    ex[:, h01, :].bitcast(BF16),
                                    start=(kt == 0),
                                    stop=(kt == NKT - 1),
                                )

                        for kt in range(NKT):
                            ss = psS.tile([P, KG, 512], F32, tag="ss", name="ss")
                            for h01 in range(2):
                                kb = h01 * 64
                                nc.tensor.matmul(
                                    ss[:, h01, :],
                                    KT[kb : kb + 32, hp, :, kt * P : (kt + 1) * P],
                                    QT[
                                        kb : kb + 32,
                                        hp,
                                        :,
                                        qt * 512 : (qt + 1) * 512,
                                    ],
                                    start=True,
                                    stop=True,
                                    perf_mode=DR,
                                )
                            ex = pde.tile([P, KG, 512], I16, tag="ex", name="ex")
                            if kt % 16 in (1, 3, 5, 7, 9, 11, 13):
                                nc.vector.tensor_scalar(
                                    ex,
                                    ss,
                                    EXP_A,
                                    EXP_B,
                                    op0=ALU.mult,
                                    op1=ALU.add,
                                )
                            else:
                                nc.scalar.activation(
                                    ex[:, :, :].bitcast(BF16),
                                    ss,
                                    AF.Exp,
                                    scale=EXP_SCALE,
                                )
                            # pv matmuls deferred two kts so the PE queue
                            # never parks behind a pending exp
                            pending.append((kt, ex))
                            if len(pending) > 4:
                                pv_mms(*pending.pop(0))
                        for it in pending:
                            pv_mms(*it)
                        pending.clear()
                        for h01 in range(2):
                            rc64 = pdr.tile([DK, 512], F32, tag="rc", name="rc")
                            nc.vector.reciprocal(
                                rc64, pvs_[h01][DK : 2 * DK, :]
                            )
                            nc.vector.tensor_tensor(
                                xo_t[:, h01, qt * 512 : (qt + 1) * 512],
                                pvs_[h01][0:DK, :],
                                rc64,
                                op=ALU.mult,
                            )

                kproj(0)
                for hp in range(NHP):
                    if hp + 1 < NHP:
                        kproj(hp + 1)
                    attn2(hp)

            pdx_cm.__exit__(None, None, None)
            psg_cm.__exit__(None, None, None)

        # ---- Phase E: out = LN(x_o @ Wo^T + bo + q)  (bo pre-added to qres)
        with (
            tc.tile_pool(name="peq", bufs=4) as peq,
            tc.tile_pool(name="pey", bufs=4) as pey,
            tc.tile_pool(name="pst", bufs=8) as pst,
            tc.tile_pool(name="psE", bufs=6, space="PSUM") as psE,
        ):
            for rt in range(NRT_O):
                qr = peq.tile([P, D], BF16)
                nc.sync.dma_start(qr, qres[rt * P : (rt + 1) * P, :])
                y = pey.tile([P, D], F32)
                for o2 in range(2):
                    ps = psE.tile([P, 512], F32)
                    for hp in range(NOT):
                        nc.tensor.matmul(
                            ps,
                            XO[hp][:, :, rt * P : (rt + 1) * P],
                            wo[hp][:, :, o2 * 512 : (o2 + 1) * 512],
                            start=(hp == 0),
                            stop=(hp == NOT - 1),
                            perf_mode=DR,
                        )
                    nc.scalar.activation(
                        y[:, o2 * 512 : (o2 + 1) * 512],
                        ps,
                        AF.Copy,
                        scale=1.0 / 2048.0,
                    )
                    aeng = nc.vector if o2 == 0 else nc.gpsimd
                    aeng.tensor_tensor(
                        y[:, o2 * 512 : (o2 + 1) * 512],
                        y[:, o2 * 512 : (o2 + 1) * 512],
                        qr[:, o2 * 512 : (o2 + 1) * 512],
                        op=ALU.add,
                    )
                stats = pst.tile([P, 2, 6], F32)
                for sg in range(2):
                    nc.vector.bn_stats(
                        stats[:, sg, :], y[:, sg * 512 : (sg + 1) * 512]
                    )
                mv = pst.tile([P, 2], F32)
                nc.vector.bn_aggr(mv, stats)
                std = pst.tile([P, 1], F32)
                nc.scalar.activation(std, mv[:, 1:2], AF.Sqrt, bias=eps_t)
                rstd = pst.tile([P, 1], F32)
                nc.vector.reciprocal(rstd, std)
                nc.gpsimd.tensor_scalar(
                    y,
                    y,
                    mv[:, 0:1],
                    rstd,
                    op0=ALU.subtract,
                    op1=ALU.mult,
                )
                eng = nc.vector if rt % 2 == 0 else nc.gpsimd
                eng.tensor_mul(y, y, g_b)
                eng.tensor_add(y, y, b_b)
                nc.sync.dma_start(out[rt * P : (rt + 1) * P, :], y)
        pec_cm.__exit__(None, None, None)
        pwo_cm.__exit__(None, None, None)
        pxo_cm.__exit__(None, None, None)
        loop_cm.__exit__(None, None, None)
    _split_sync_waits(nc)
    return nc


_NC = None


def _get_nc():
    global _NC
    if _NC is None:
        _NC = build_nc()
    return _NC


def _split_dk(a):
    """[D, N] -> [64, NDT, 2, N] with d = dt*128 + i*64 + p."""
    Dd, N = a.shape
    return np.ascontiguousarray(
        a.reshape(NDT, 2, DK, N).transpose(2, 0, 1, 3)
    )


def prepare_in_maps(q, k, v, Wq, bq, Wk, bk, Wv, bv, Wo, bo, ln_g, ln_b):
    f = np.float32
    f8 = ml_dtypes.float8_e4m3
    q = np.asarray(q, f)
    k = np.asarray(k, f)
    v = np.asarray(v, f)
    wq8 = _split_dk(np.asarray(Wq, f).T * 32.0).astype(f8)
    wk8 = _split_dk(np.asarray(Wk, f).T * 32.0).astype(f8)
    wv8 = _split_dk(np.asarray(Wv, f).T * 32.0).astype(f8)
    wo8 = _split_dk(np.asarray(Wo, f).T * 32.0).astype(f8)
    common = {
        "wq8": wq8,
        "wk8": wk8,
        "wv8": wv8,
        "wo8": wo8,
        "bq32": np.ascontiguousarray((np.asarray(bq, f) * 32.0).reshape(NOT, P).T),
        "bk32": np.ascontiguousarray((np.asarray(bk, f) * 32.0).reshape(NOT, P).T),
        "bkr": (np.asarray(bk, f) * 32.0).reshape(NHP, P).astype(ml_dtypes.bfloat16),
        "bvb": (np.asarray(bv, f) * 32.0).astype(ml_dtypes.bfloat16),
        "ln_g": np.asarray(ln_g, f),
        "ln_b": np.asarray(ln_b, f),
    }
    in_maps = []
    for c in range(8):
        b_, half = divmod(c, 2)
        qs = q[b_, half * M : (half + 1) * M, :]
        qres_c = (qs + np.asarray(bo, f)[None, :]).astype(ml_dtypes.bfloat16)
        in_maps.append(
            dict(
                common,
                xq8=_split_dk(qs.T).astype(f8),
                xk8=_split_dk(k[b_].T).astype(f8),
                xv8=_split_dk(v[b_].T).astype(f8),
                qres=np.ascontiguousarray(qres_c),
            )
        )
    return in_maps


def kernel(q, k, v, Wq, bq, Wk, bk, Wv, bv, Wo, bo, ln_g, ln_b):
    nc = _get_nc()
    in_maps = prepare_in_maps(q, k, v, Wq, bq, Wk, bk, Wv, bv, Wo, bo, ln_g, ln_b)
    res = run_bass_kernel_spmd(nc, in_maps, core_ids=list(range(8)))
    out = np.empty((B, S, D), np.float32)
    for c in range(8):
        b_, half = divmod(c, 2)
        out[b_, half * M : (half + 1) * M, :] = res.results[c]["out"]
    return out
